# revision 1
# baseline (speedup 1.0000x reference)
"""Trainium2 Bass kernel for nn_GAT_80118319940093 (2-layer GAT + mean-pool).

Self-contained: takes FULL inputs, shards across 8 NeuronCores internally.

v2 architecture (per core, SPMD single program, dst-node sharding):
  kernel1 (layer 1):
    phase A: haug[n] = [h fp8e4(128) | al_src bf16(8)] rows (144B of a 256B
             row) for ALL nodes into the core's own HBM table, batched
             8 tiles per DMA / 4 tiles per PSUM convert.  An own-nodes pass
             (xTo) additionally keeps h_own (fp8), al_src/al_dst (bf16)
             resident in SBUF.
    phase B: edges (NO self loops), sorted by (dst block, src table), with
             per-(block,table) bucket sizes fixed at compile time to
             roundup128(max over cores of the true count).  Per (wave,
             table) run: one dma_gather, one S-matrix build (single
             broadcast tensor_tensor), ST one-hot via PE ones-broadcast +
             DVE is_equal reading PSUM directly, per-group aldexp matmuls,
             exp(leakyrelu) in 2 ops, fused scatter matmul (msg|w).
             Self-loop contributions are injected per wave from resident
             own h/als/ald via an identity matmul (start=True).
  host relay: concat h2aug shards -> 256B-row table h2t + own4 per core.
  kernel2 (layer 2 + pooling): same edge geometry, gathers h2t[src],
             same S/ST structure with H=1, scatter (2ch + denom), per-graph
             partial sums via a host-supplied one-hot batch matrix.
  host epilogue: sum 8 partials, divide by graph counts, log_softmax.
"""
import os
import sys

sys.path.insert(0, "/opt/trn_rl_repo")

import numpy as np
import ml_dtypes

import concourse.bass as bass
import concourse.bacc as bacc
import concourse.mybir as mybir
import concourse.tile as tile

F32 = mybir.dt.float32
BF16 = mybir.dt.bfloat16
FP8 = mybir.dt.float8e4
I16 = mybir.dt.int16
U8 = mybir.dt.uint8
AOT = mybir.AluOpType
ACT_F = mybir.ActivationFunctionType
P = 128

NEG_SLOPE = 0.2
ALS_OFF = 64  # al_src offset in bf16 units within a 256B row (=128 u8)
PAD_DC = 300.0


class Cfg:
    def __init__(self, N, E, F_IN, HEADS, C1, NUM_GRAPHS, CORES, WAVE_BLOCKS,
                 N_TABLES):
        self.N, self.E, self.F_IN = N, E, F_IN
        self.HEADS, self.C1 = HEADS, C1
        self.NUM_GRAPHS, self.CORES = NUM_GRAPHS, CORES
        self.WB = WAVE_BLOCKS               # blocks per wave
        self.NT = N_TABLES
        per_core = -(-N // CORES)
        blocks = -(-per_core // P)
        self.NWAVE = -(-blocks // self.WB)
        self.BLOCKS = self.NWAVE * self.WB  # incl pad blocks
        self.NODES_PC = self.BLOCKS * P     # padded nodes per core
        self.NPAD = self.NODES_PC * CORES   # padded total nodes
        assert self.NPAD % N_TABLES == 0
        self.TROWS = self.NPAD // N_TABLES  # gather-table rows
        assert self.TROWS <= 32768, "int16 gather index limit"
        self.NTILE = self.NPAD // P         # phase-A tiles


def make_cfg_full():
    return Cfg(N=100000, E=1600000, F_IN=128, HEADS=8, C1=16, NUM_GRAPHS=64,
               CORES=8, WAVE_BLOCKS=4, N_TABLES=4)


class Plan:
    """Static (shared across cores) edge-stream layout, sized from data."""

    def __init__(self, L_bt, cfg):
        # L_bt: [BLOCKS][NT] slot counts, each a multiple of 128
        self.L_bt = L_bt
        runs = []
        slot_base = 0
        grp_base = 0
        for w in range(cfg.NWAVE):
            for t in range(cfg.NT):
                blks = list(range(w * cfg.WB, (w + 1) * cfg.WB))
                Ls = [L_bt[b][t] for b in blks]
                L_run = sum(Ls)
                G_run = L_run // P
                gblocks = []
                for b, Lb in zip(blks, Ls):
                    gblocks += [b] * (Lb // P)
                runs.append(dict(w=w, t=t, L=L_run, G=G_run,
                                 slot_base=slot_base, grp_base=grp_base,
                                 gblocks=gblocks))
                slot_base += L_run
                grp_base += G_run
        self.runs = runs
        self.SLOTS = slot_base
        self.GTOT = grp_base
        self.Gmax = max((r["G"] for r in runs), default=0)
        # per-wave aggregates
        self.wave_L = [sum(r["L"] for r in runs[w * cfg.NT:(w + 1) * cfg.NT])
                       for w in range(cfg.NWAVE)]
        self.wave_G = [sum(r["G"] for r in runs[w * cfg.NT:(w + 1) * cfg.NT])
                       for w in range(cfg.NWAVE)]
        self.GWmax = max(self.wave_G)
        self.LWmax = max(self.wave_L)
        self.Lmax = max((r["L"] for r in runs), default=0)
        # last (t, gi) touching each block, for matmul stop flags
        self.last_touch = {}
        for r in runs:
            for gi, b in enumerate(r["gblocks"]):
                self.last_touch[b] = (r["t"], gi)

    def key(self):
        return hash(tuple(tuple(row) for row in self.L_bt))


# ---------------------------------------------------------------- host prep

def fold_weights(W1, a_src1, a_dst1, W2, a_src2, a_dst2, cfg):
    """Folded weights with the h-feature axis permuted to c-major order
    (feature (h, c) stored at column c*H + h) so the per-edge w-broadcast
    multiply has its broadcast on a non-last axis (DVE 4x mode)."""
    H, C = cfg.HEADS, cfg.C1
    vsrc = np.zeros((cfg.F_IN, H), np.float32)
    vdst = np.zeros((cfg.F_IN, H), np.float32)
    for h in range(H):
        vsrc[:, h] = W1[:, h * C:(h + 1) * C] @ a_src1[h]
        vdst[:, h] = W1[:, h * C:(h + 1) * C] @ a_dst1[h]
    # c-major permutation of the 128 h-columns
    perm = (np.arange(C)[:, None] * H + np.arange(H)[None, :])  # [C, H]
    inv = np.empty(H * C, np.int64)
    inv[perm.reshape(-1)] = (np.arange(H)[None, :] * C
                             + np.arange(C)[:, None]).reshape(-1)
    W1cm = W1[:, inv]
    W1aug = np.concatenate([W1cm, vsrc, vdst], 1)        # [F, H*C+2H]
    vs2 = W2 @ a_src2[0]
    vd2 = W2 @ a_dst2[0]
    W2aug = np.concatenate([W2, vs2[:, None], vd2[:, None]], 1)  # [HC, 4]
    W2aug = W2aug[inv, :]
    return W1aug.astype(ml_dtypes.bfloat16), W2aug.astype(ml_dtypes.bfloat16)


def prep_edges(edge_index, cfg):
    """Bucket real edges (no self loops) per core by (dst block, src table).

    Returns (plan, cores) where cores[k] = dict(idx, dcolb, drow)."""
    src = edge_index[0].astype(np.int32)
    dst = edge_index[1].astype(np.int32)
    core = dst // cfg.NODES_PC
    blk = (dst % cfg.NODES_PC) >> 7
    tbl = src // cfg.TROWS
    counts = np.zeros((cfg.CORES, cfg.BLOCKS, cfg.NT), np.int64)
    np.add.at(counts, (core, blk, tbl), 1)
    mx = counts.max(axis=0)
    L_bt = (((mx + P - 1) // P) * P).astype(np.int64)
    plan = Plan([[int(L_bt[b, t]) for t in range(cfg.NT)]
                 for b in range(cfg.BLOCKS)], cfg)

    # per-core slot streams
    key_all = ((core * cfg.BLOCKS + blk) * cfg.NT + tbl).astype(np.int64)
    order = np.argsort(key_all, kind="stable")
    key_s = key_all[order]
    bounds = np.searchsorted(
        key_s, np.arange(cfg.CORES * cfg.BLOCKS * cfg.NT + 1))
    loc_all = (src - tbl * cfg.TROWS).astype(np.int16)
    dloc_all = (dst % cfg.NODES_PC) & 127

    # bucket slot offsets within the stream (shared layout)
    bucket_off = {}
    for r in plan.runs:
        off = r["slot_base"]
        for b in range(r["w"] * cfg.WB, (r["w"] + 1) * cfg.WB):
            bucket_off[(b, r["t"])] = off
            off += plan.L_bt[b][r["t"]]

    cores = []
    for k in range(cfg.CORES):
        idx = np.zeros(plan.SLOTS, np.int16)
        dcol = np.full(plan.SLOTS, PAD_DC, np.float32)
        for b in range(cfg.BLOCKS):
            for t in range(cfg.NT):
                a = bounds[(k * cfg.BLOCKS + b) * cfg.NT + t]
                z = bounds[(k * cfg.BLOCKS + b) * cfg.NT + t + 1]
                n = z - a
                assert n <= plan.L_bt[b][t]
                sel = order[a:z]
                o = bucket_off[(b, t)]
                idx[o:o + n] = loc_all[sel]
                dcol[o:o + n] = dloc_all[sel]
        # idx wrapped per run: slot i -> [i%16, i//16], tiled to 128 rows
        idx_w = np.empty((16, plan.SLOTS // 16), np.int16)
        pos = 0
        for r in plan.runs:
            L = r["L"]
            seg = idx[r["slot_base"]:r["slot_base"] + L]
            idx_w[:, pos:pos + L // 16] = seg.reshape(-1, 16).T
            pos += L // 16
        idx_w = np.tile(idx_w, (8, 1))
        # dcol in [128, GTOT] layout (slot i -> [i%128, grp_base + i//128])
        dcb = dcol.reshape(-1, P).T.astype(ml_dtypes.bfloat16)
        drow = dcol.astype(ml_dtypes.bfloat16).reshape(1, plan.SLOTS)
        cores.append(dict(idx=np.ascontiguousarray(idx_w),
                          dcolb=np.ascontiguousarray(dcb),
                          drow=np.ascontiguousarray(drow)))
    return plan, cores


def prep_bsel(batch, cfg):
    """Per-core one-hot [128, BLOCKS*NUM_GRAPHS] bf16 batch-selection."""
    G = cfg.NUM_GRAPHS
    b_full = np.full(cfg.NPAD, -1, np.int64)
    b_full[:cfg.N] = batch
    out = []
    for k in range(cfg.CORES):
        seg = b_full[k * cfg.NODES_PC:(k + 1) * cfg.NODES_PC].reshape(-1, P)
        oh = (seg[:, :, None] == np.arange(G)[None, None, :])
        sel = np.ascontiguousarray(
            np.transpose(oh, (1, 0, 2)).reshape(P, cfg.BLOCKS * G))
        out.append(sel.astype(ml_dtypes.bfloat16))
    return out


# ------------------------------------------------------------- bass builders

def build_kernel1(cfg, plan):
    nc = bacc.Bacc("TRN2", target_bir_lowering=False, debug=False)
    H = cfg.HEADS
    C = cfg.C1
    FOUT = H * cfg.C1            # 128
    NAUG = FOUT + 2 * H          # 144
    NB = FOUT + H                # 136
    xT = nc.dram_tensor("xT", [cfg.F_IN, cfg.NPAD], BF16,
                        kind="ExternalInput").ap()
    xTo = nc.dram_tensor("xTo", [cfg.F_IN, cfg.NODES_PC], BF16,
                         kind="ExternalInput").ap()
    W1a = nc.dram_tensor("W1a", [cfg.F_IN, NAUG], BF16,
                         kind="ExternalInput").ap()
    W2a = nc.dram_tensor("W2a", [FOUT, 4], BF16, kind="ExternalInput").ap()
    idx = nc.dram_tensor("idx", [P, plan.SLOTS // 16], I16,
                         kind="ExternalInput").ap()
    dcolb = nc.dram_tensor("dcolb", [P, plan.GTOT], BF16,
                           kind="ExternalInput").ap()
    drow = nc.dram_tensor("drow", [1, plan.SLOTS], BF16,
                          kind="ExternalInput").ap()
    ones1 = nc.dram_tensor("ones1", [1, P], BF16, kind="ExternalInput").ap()
    iotab = nc.dram_tensor("iotab", [P, P * plan.Gmax], BF16,
                           kind="ExternalInput").ap()
    iotac = nc.dram_tensor("iotac", [P, 1], F32, kind="ExternalInput").ap()
    identf = nc.dram_tensor("identf", [P, P], F32, kind="ExternalInput").ap()
    identb = nc.dram_tensor("identb", [P, P], BF16, kind="ExternalInput").ap()
    haugs = [nc.dram_tensor(f"haug{t}", [cfg.TROWS, 512], U8,
                            kind="Internal").ap() for t in range(cfg.NT)]
    h2aug = nc.dram_tensor("h2aug", [cfg.NODES_PC, 4], F32,
                           kind="ExternalOutput").ap()

    NT, WB = cfg.NT, cfg.WB
    Gmax = plan.Gmax
    with tile.TileContext(nc) as tc:
        with tc.tile_pool(name="const", bufs=1) as cpool:
            w1_t = cpool.tile([cfg.F_IN, NAUG], BF16)
            nc.sync.dma_start(w1_t[:], W1a)
            w2_t = cpool.tile([FOUT, 4], BF16)
            nc.sync.dma_start(w2_t[:], W2a)
            ones_t = cpool.tile([1, P], BF16)
            nc.sync.dma_start(ones_t[:], ones1)
            iob = cpool.tile([P, P * plan.Gmax], BF16)
            nc.sync.dma_start(iob[:], iotab)
            ioc = cpool.tile([P, 1], F32)
            nc.sync.dma_start(ioc[:], iotac)
            idf = cpool.tile([P, P], F32)
            nc.sync.dma_start(idf[:], identf)
            idb = cpool.tile([P, P], BF16)
            nc.sync.dma_start(idb[:], identb)
            h_own = cpool.tile([P, cfg.BLOCKS * FOUT], BF16)  # c-major h
            aa_own = cpool.tile([P, cfg.BLOCKS * 16], BF16)   # als|ald
            acc_sb = cpool.tile([P, cfg.BLOCKS * NB], F32)    # block sums

            regs = {}

            def get_reg(v):
                if v not in regs:
                    regs[v] = nc.gpsimd.to_reg(v)
                return regs[v]

            with tc.tile_pool(name="pA", bufs=2) as pa, \
                 tc.tile_pool(name="pAps", bufs=1, space="PSUM") as paps, \
                 tc.tile_pool(name="pBio", bufs=3) as pio, \
                 tc.tile_pool(name="pB", bufs=3) as pb, \
                 tc.tile_pool(name="pBsm", bufs=2) as pbs, \
                 tc.tile_pool(name="scat", bufs=2, space="PSUM") as scps, \
                 tc.tile_pool(name="bc", bufs=2, space="PSUM") as bcps, \
                 tc.tile_pool(name="pBo", bufs=2) as pbo:
                # ---- own-node pass: h_own, als|ald resident ----
                for gd in range(cfg.BLOCKS // 4):
                    xt = pa.tile([cfg.F_IN, 4 * P], BF16, tag="xto")
                    nc.sync.dma_start(xt[:],
                                      xTo[:, gd * 4 * P:(gd + 1) * 4 * P])
                    ph = paps.tile([P, 4 * FOUT], F32, space="PSUM", tag="ph")
                    pa_ = paps.tile([P, 4 * 16], F32, space="PSUM", tag="pa2")
                    for t4 in range(4):
                        nc.tensor.matmul(
                            ph[:, t4 * FOUT:(t4 + 1) * FOUT],
                            lhsT=xt[:, t4 * P:(t4 + 1) * P],
                            rhs=w1_t[:, 0:FOUT], start=True, stop=True)
                        nc.tensor.matmul(
                            pa_[:, t4 * 16:(t4 + 1) * 16],
                            lhsT=xt[:, t4 * P:(t4 + 1) * P],
                            rhs=w1_t[:, FOUT:FOUT + 2 * H], start=True,
                            stop=True)
                    nc.scalar.activation(
                        h_own[:, gd * 4 * FOUT:(gd + 1) * 4 * FOUT],
                        ph[:], ACT_F.Copy)
                    nc.vector.tensor_copy(
                        aa_own[:, gd * 4 * 16:(gd + 1) * 4 * 16], pa_[:])
                # ---- self-loop contributions seed acc_sb ----
                for w in range(cfg.NWAVE):
                    zs = pbo.tile([P, WB, H], F32, tag="zs")
                    nc.vector.tensor_tensor(
                        out=zs[:],
                        in0=aa_own[:, w * WB * 16:(w + 1) * WB * 16]
                        .rearrange("p (b c) -> p b c", c=16)[:, :, 0:H],
                        in1=aa_own[:, w * WB * 16:(w + 1) * WB * 16]
                        .rearrange("p (b c) -> p b c", c=16)[:, :, H:2 * H],
                        op=AOT.add)
                    nc.vector.scalar_tensor_tensor(
                        out=zs[:], in0=zs[:], scalar=NEG_SLOPE, in1=zs[:],
                        op0=AOT.mult, op1=AOT.max)
                    msf = pbo.tile([P, WB, NB], F32, tag="msf")
                    nc.scalar.activation(
                        msf[:, :, FOUT:FOUT + H], zs[:], ACT_F.Exp)
                    nc.vector.tensor_tensor(
                        out=msf[:, :, 0:FOUT].rearrange(
                            "p b (c h) -> p b c h", h=H),
                        in0=h_own[:, w * WB * FOUT:(w + 1) * WB * FOUT]
                        .rearrange("p (b c h) -> p b c h", b=WB, h=H),
                        in1=msf[:, :, FOUT:FOUT + H]
                        .rearrange("p b (one h) -> p b one h", one=1)
                        .to_broadcast([P, WB, cfg.C1, H]),
                        op=AOT.mult)
                    nc.vector.tensor_copy(
                        acc_sb[:, w * WB * NB:(w + 1) * WB * NB], msf[:])

                # ---- table-major: write haug[t], then its runs ----
                RPT = cfg.TROWS // (8 * P)   # 8-tile write groups per table
                for t in range(NT):
                    for gd in range(RPT):
                        gt = t * RPT + gd
                        xt = pa.tile([cfg.F_IN, 8 * P], BF16, tag="xt")
                        nc.sync.dma_start(
                            xt[:], xT[:, gt * 8 * P:(gt + 1) * 8 * P])
                        row = pa.tile([P, 8, 512], U8, tag="row")
                        rb = row[:].bitcast(BF16)   # [P, 8, 256]
                        for q in range(2):
                            ph = paps.tile([P, 4 * FOUT], F32, space="PSUM",
                                           tag="ph")
                            pa_ = paps.tile([P, 4 * 16], F32, space="PSUM",
                                            tag="pa2")
                            for t4 in range(4):
                                tt = q * 4 + t4
                                nc.tensor.matmul(
                                    ph[:, t4 * FOUT:(t4 + 1) * FOUT],
                                    lhsT=xt[:, tt * P:(tt + 1) * P],
                                    rhs=w1_t[:, 0:FOUT], start=True,
                                    stop=True)
                                nc.tensor.matmul(
                                    pa_[:, t4 * 16:(t4 + 1) * 16],
                                    lhsT=xt[:, tt * P:(tt + 1) * P],
                                    rhs=w1_t[:, FOUT:FOUT + 2 * H],
                                    start=True, stop=True)
                            nc.scalar.activation(
                                rb[:, q * 4:(q + 1) * 4, 0:FOUT],
                                ph[:].rearrange("p (t c) -> p t c", c=FOUT),
                                ACT_F.Copy)
                            nc.vector.tensor_copy(
                                rb[:, q * 4:(q + 1) * 4, FOUT:FOUT + H],
                                pa_[:].rearrange("p (t c) -> p t c", c=16)
                                [:, :, 0:H])
                        nc.sync.dma_start(
                            haugs[t][gd * 8 * P:(gd + 1) * 8 * P, 0:272]
                            .rearrange("(t p) c -> p t c", p=P),
                            row[:, :, 0:272])
                    # ---- all waves' runs for this table ----
                    for w in range(cfg.NWAVE):
                        r = plan.runs[w * NT + t]
                        L, G = r["L"], r["G"]
                        if G == 0:
                            continue
                        sb, gbase = r["slot_base"], r["grp_base"]
                        # ST path first (independent of the gather)
                        drw = pbs.tile([1, Gmax * P], BF16, tag="drw")
                        nc.sync.dma_start(drw[:, 0:L], drow[:, sb:sb + L])
                        dct = pio.tile([P, Gmax], BF16, tag="dct")
                        nc.sync.dma_start(dct[:, 0:G],
                                          dcolb[:, gbase:gbase + G])
                        ST = pbs.tile([P, Gmax * P], BF16, tag="ST")
                        for ci, pc in enumerate(range(0, G * P, 512)):
                            pw = min(512, G * P - pc)
                            bcp = bcps.tile([P, 512], F32, space="PSUM",
                                            tag="bcp")
                            nc.tensor.matmul(
                                bcp[:, :pw], lhsT=ones_t[:],
                                rhs=drw[0:1, pc:pc + pw],
                                start=True, stop=True)
                            if True:  # all ST chunks Act-assisted
                                dcs = pbs.tile([P, 512], BF16, tag="dcs")
                                nc.scalar.activation(dcs[:, :pw],
                                                     bcp[:, :pw], ACT_F.Copy)
                                nc.vector.tensor_scalar(
                                    out=ST[:, pc:pc + pw], in0=dcs[:, :pw],
                                    scalar1=ioc[:, :1], scalar2=None,
                                    op0=AOT.is_equal)
                            else:
                                nc.vector.tensor_scalar(
                                    out=ST[:, pc:pc + pw], in0=bcp[:, :pw],
                                    scalar1=ioc[:, :1], scalar2=None,
                                    op0=AOT.is_equal)
                        # S one-hot [dstcol, slotgroup] in one 4x op
                        S = pbs.tile([P, P, Gmax], BF16, tag="S")
                        nc.vector.tensor_tensor(
                            out=S[:, :, 0:G],
                            in0=iob[:].rearrange("p (c g) -> p c g",
                                                 g=Gmax)[:, :, 0:G],
                            in1=dct[:, 0:G]
                            .rearrange("p (one g) -> p one g", one=1)
                            .to_broadcast([P, P, G]),
                            op=AOT.is_equal)
                        # aldexp per group
                        axp = bcps.tile([P, Gmax * H], F32, space="PSUM",
                                        tag="axp")
                        for gi in range(G):
                            b = r["gblocks"][gi]
                            nc.tensor.matmul(
                                axp[:, gi * H:(gi + 1) * H],
                                lhsT=ST[:, gi * P:(gi + 1) * P],
                                rhs=aa_own[:, b * 16 + H:b * 16 + 2 * H],
                                start=True, stop=True)
                        # gather
                        idxw = pio.tile([P, plan.Lmax // 16], I16,
                                        tag="idxw")
                        nc.sync.dma_start(idxw[:, 0:L // 16],
                                          idx[:, sb // 16:(sb + L) // 16])
                        g = pb.tile([P, Gmax, 512], U8, tag="g")
                        nc.gpsimd.dma_gather(
                            g[:, 0:G, :], haugs[t],
                            idxw[:, 0:L // 16], L, get_reg(L),
                            512, single_packet=False)
                        gb = g[:].bitcast(BF16)       # [P, Gmax, 256]
                        # w = exp(leakyrelu(als + aldexp))
                        z = pbs.tile([P, Gmax, H], F32, tag="z")
                        nc.vector.tensor_tensor(
                            out=z[:, 0:G, :],
                            in0=gb[:, 0:G, FOUT:FOUT + H],
                            in1=axp[:, 0:G * H]
                            .rearrange("p (g h) -> p g h", h=H),
                            op=AOT.add)
                        nc.vector.scalar_tensor_tensor(
                            out=z[:, 0:G, :], in0=z[:, 0:G, :],
                            scalar=NEG_SLOPE, in1=z[:, 0:G, :],
                            op0=AOT.mult, op1=AOT.max)
                        msg = pbs.tile([P, Gmax, NB], BF16, tag="msg")
                        nc.scalar.activation(msg[:, 0:G, FOUT:FOUT + H],
                                             z[:, 0:G, :], ACT_F.Exp)
                        nc.vector.tensor_tensor(
                            out=msg[:, 0:G, 0:FOUT].rearrange(
                                "p g (c h) -> p g c h", h=H),
                            in0=gb[:, 0:G, 0:FOUT].rearrange(
                                "p g (c h) -> p g c h", h=H),
                            in1=msg[:, 0:G, FOUT:FOUT + H]
                            .rearrange("p g (one h) -> p g one h", one=1)
                            .to_broadcast([P, G, cfg.C1, H]),
                            op=AOT.mult)
                        # scatter per block segment into a rotating
                        # PSUM bank, then fold into the SBUF accumulator
                        gi = 0
                        while gi < G:
                            b = r["gblocks"][gi]
                            ge = gi
                            while ge < G and r["gblocks"][ge] == b:
                                ge += 1
                            pacc = scps.tile([P, NB], F32, space="PSUM",
                                             tag="acc")
                            for gj in range(gi, ge):
                                nc.tensor.matmul(
                                    pacc[:],
                                    lhsT=S[:, :, gj], rhs=msg[:, gj, :],
                                    start=(gj == gi), stop=(gj == ge - 1),
                                    skip_group_check=True)
                            nc.vector.tensor_tensor(
                                out=acc_sb[:, b * NB:(b + 1) * NB],
                                in0=acc_sb[:, b * NB:(b + 1) * NB],
                                in1=pacc[:], op=AOT.add)
                            gi = ge

                # ---------------- batched epilogue ------------------------
                for w in range(cfg.NWAVE):
                    h2w = pbo.tile([P, WB, 4], F32, tag="h2w")
                    rec = pbo.tile([P, WB, H], F32, tag="rec")
                    eluw = pbo.tile([P, WB, FOUT], F32, tag="eluw")
                    negw = pbo.tile([P, WB, FOUT], F32, tag="negw")
                    av = acc_sb[:, w * WB * NB:(w + 1) * WB * NB].rearrange(
                        "p (b n) -> p b n", n=NB)
                    nc.vector.tensor_scalar(
                        out=rec[:], in0=av[:, :, FOUT:FOUT + H],
                        scalar1=1e-12, scalar2=None, op0=AOT.add)
                    nc.vector.reciprocal(rec[:], rec[:])
                    nc.vector.tensor_tensor(
                        out=eluw[:].rearrange("p b (c h) -> p b c h", h=H),
                        in0=av[:, :, 0:FOUT].rearrange(
                            "p b (c h) -> p b c h", h=H),
                        in1=rec[:].rearrange("p b (one h) -> p b one h",
                                             one=1)
                        .to_broadcast([P, WB, cfg.C1, H]),
                        op=AOT.mult)
                    nc.vector.tensor_scalar(out=negw[:], in0=eluw[:],
                                            scalar1=0.0, scalar2=None,
                                            op0=AOT.min)
                    nc.scalar.activation(negw[:], negw[:], ACT_F.Exp)
                    nc.vector.scalar_tensor_tensor(
                        out=eluw[:], in0=eluw[:], scalar=0.0, in1=negw[:],
                        op0=AOT.max, op1=AOT.add)
                    nc.vector.tensor_scalar(out=eluw[:], in0=eluw[:],
                                            scalar1=1.0, scalar2=None,
                                            op0=AOT.subtract)
                    for b in range(WB):
                        eluT_ps = bcps.tile([P, 512], F32, space="PSUM",
                                            tag="bcp")
                        nc.tensor.transpose(eluT_ps[:, :P], eluw[:, b, :],
                                            idf[:])
                        eluT = pbo.tile([P, P], BF16, tag="eluTs")
                        nc.scalar.activation(eluT[:], eluT_ps[:, :P],
                                             ACT_F.Copy)
                        h2ps = bcps.tile([P, 512], F32, space="PSUM",
                                         tag="bcp")
                        nc.tensor.matmul(h2ps[:, :4], lhsT=eluT[:],
                                         rhs=w2_t[:], start=True, stop=True)
                        nc.vector.tensor_copy(h2w[:, b, :], h2ps[:, :4])
                    nc.sync.dma_start(
                        h2aug[w * WB * P:(w + 1) * WB * P, :]
                        .rearrange("(b p) c -> p b c", p=P),
                        h2w[:])
    nc.compile()
    return nc


def build_kernel2(cfg, plan):
    nc = bacc.Bacc("TRN2", target_bir_lowering=False, debug=False)
    G = cfg.NUM_GRAPHS
    NT, WB = cfg.NT, cfg.WB
    h2t = nc.dram_tensor("h2t", [cfg.NPAD, 64], F32,
                         kind="ExternalInput").ap()
    own4 = nc.dram_tensor("own4", [P, cfg.BLOCKS * 4], F32,
                          kind="ExternalInput").ap()
    idx = nc.dram_tensor("idx", [P, plan.SLOTS // 16], I16,
                         kind="ExternalInput").ap()
    dcolb = nc.dram_tensor("dcolb", [P, plan.GTOT], BF16,
                           kind="ExternalInput").ap()
    drow = nc.dram_tensor("drow", [1, plan.SLOTS], BF16,
                          kind="ExternalInput").ap()
    ones1 = nc.dram_tensor("ones1", [1, P], BF16, kind="ExternalInput").ap()
    iotab = nc.dram_tensor("iotab", [P, P * plan.Gmax], BF16,
                           kind="ExternalInput").ap()
    iotac = nc.dram_tensor("iotac", [P, 1], F32, kind="ExternalInput").ap()
    identb = nc.dram_tensor("identb", [P, P], BF16, kind="ExternalInput").ap()
    bsel = nc.dram_tensor("bsel", [P, cfg.BLOCKS * G], BF16,
                          kind="ExternalInput").ap()
    part = nc.dram_tensor("part", [G, 2], F32, kind="ExternalOutput").ap()

    with tile.TileContext(nc) as tc:
        with tc.tile_pool(name="const", bufs=1) as cpool:
            ones_t = cpool.tile([1, P], BF16)
            nc.sync.dma_start(ones_t[:], ones1)
            iob = cpool.tile([P, P * plan.Gmax], BF16)
            nc.sync.dma_start(iob[:], iotab)
            ioc = cpool.tile([P, 1], F32)
            nc.sync.dma_start(ioc[:], iotac)
            idb = cpool.tile([P, P], BF16)
            nc.sync.dma_start(idb[:], identb)
            own_t = cpool.tile([P, cfg.BLOCKS * 4], F32)
            nc.sync.dma_start(own_t[:], own4)
            sel_t = cpool.tile([P, cfg.BLOCKS * G], BF16)
            nc.sync.dma_start(sel_t[:], bsel)
            ald2 = cpool.tile([P, cfg.BLOCKS], BF16)
            nc.vector.tensor_copy(
                ald2[:],
                own_t[:].rearrange("p (b c) -> p b c", c=4)[:, :, 3])

            Gmax = plan.Gmax
            regs = {}

            def get_reg(v):
                if v not in regs:
                    regs[v] = nc.gpsimd.to_reg(v)
                return regs[v]

            with tc.tile_pool(name="pBio", bufs=2) as pio, \
                 tc.tile_pool(name="pB", bufs=3) as pb, \
                 tc.tile_pool(name="pBsm", bufs=2) as pbs, \
                 tc.tile_pool(name="scat", bufs=1, space="PSUM") as scps, \
                 tc.tile_pool(name="bc", bufs=2, space="PSUM") as bcps, \
                 tc.tile_pool(name="axps", bufs=1, space="PSUM") as axps, \
                 tc.tile_pool(name="pBo", bufs=3) as pbo, \
                 tc.tile_pool(name="pool", bufs=1, space="PSUM") as plps:
                gacc = plps.tile([G, 2], F32, space="PSUM")
                for w in range(cfg.NWAVE):
                    rbase = w * NT
                    wave_runs = plan.runs[rbase:rbase + NT]
                    Lw, Gw = plan.wave_L[w], plan.wave_G[w]
                    sb0 = wave_runs[0]["slot_base"]
                    gb0 = wave_runs[0]["grp_base"]
                    idxw = pio.tile([P, plan.LWmax // 16], I16, tag="idxw")
                    nc.sync.dma_start(
                        idxw[:, 0:Lw // 16],
                        idx[:, sb0 // 16:(sb0 + Lw) // 16])
                    dct = pio.tile([P, plan.GWmax], BF16, tag="dct")
                    nc.sync.dma_start(dct[:, 0:Gw], dcolb[:, gb0:gb0 + Gw])

                    ow = own_t[:].rearrange("p (b c) -> p b c", c=4)
                    zs = pbo.tile([P, WB], F32, tag="zs")
                    nc.vector.tensor_tensor(
                        out=zs[:], in0=ow[:, w * WB:(w + 1) * WB, 2],
                        in1=ow[:, w * WB:(w + 1) * WB, 3], op=AOT.add)
                    nc.vector.scalar_tensor_tensor(
                        out=zs[:], in0=zs[:], scalar=NEG_SLOPE, in1=zs[:],
                        op0=AOT.mult, op1=AOT.max)
                    wsf = pbo.tile([P, WB], BF16, tag="wsf")
                    nc.scalar.activation(wsf[:], zs[:], ACT_F.Exp)
                    msf = pbo.tile([P, WB, 3], BF16, tag="msf")
                    nc.vector.tensor_tensor(
                        out=msf[:, :, 0:2],
                        in0=ow[:, w * WB:(w + 1) * WB, 0:2],
                        in1=wsf[:].rearrange("p (b one) -> p b one", one=1)
                        .to_broadcast([P, WB, 2]),
                        op=AOT.mult)
                    nc.vector.tensor_copy(msf[:, :, 2], wsf[:])
                    accs = []
                    for b in range(WB):
                        acc_t = scps.tile([P, 3], F32, space="PSUM",
                                          tag=f"acc{b}")
                        accs.append(acc_t)

                    def acc_ap(bb):
                        return accs[bb][:]

                    for b in range(WB):
                        blk = w * WB + b
                        lt = plan.last_touch.get(blk)
                        nc.tensor.matmul(acc_ap(b), lhsT=idb[:],
                                         rhs=msf[:, b, :], start=True,
                                         stop=(lt is None),
                                         skip_group_check=True)

                    for t in range(NT):
                        r = plan.runs[rbase + t]
                        L, Gr = r["L"], r["G"]
                        if Gr == 0:
                            continue
                        io_off = (r["slot_base"] - sb0) // 16
                        g_off = r["grp_base"] - gb0
                        drw = pbs.tile([1, Gmax * P], BF16, tag="drw")
                        nc.sync.dma_start(
                            drw[:, 0:L],
                            drow[:, r["slot_base"]:r["slot_base"] + L])
                        ST = pbs.tile([P, Gmax * P], BF16, tag="ST")
                        for pc in range(0, Gr * P, 512):
                            pw = min(512, Gr * P - pc)
                            bcp = bcps.tile([P, 512], F32, space="PSUM",
                                            tag="bcp")
                            nc.tensor.matmul(
                                bcp[:, :pw], lhsT=ones_t[:],
                                rhs=drw[0:1, pc:pc + pw],
                                start=True, stop=True)
                            if (pc // 512) % 2 == 0:
                                dcs = pbs.tile([P, 512], BF16, tag="dcs")
                                nc.scalar.activation(dcs[:, :pw],
                                                     bcp[:, :pw], ACT_F.Copy)
                                nc.vector.tensor_scalar(
                                    out=ST[:, pc:pc + pw], in0=dcs[:, :pw],
                                    scalar1=ioc[:, :1], scalar2=None,
                                    op0=AOT.is_equal)
                            else:
                                nc.vector.tensor_scalar(
                                    out=ST[:, pc:pc + pw], in0=bcp[:, :pw],
                                    scalar1=ioc[:, :1], scalar2=None,
                                    op0=AOT.is_equal)
                        S = pbs.tile([P, P, Gmax], BF16, tag="S")
                        nc.vector.tensor_tensor(
                            out=S[:, :, 0:Gr],
                            in0=iob[:].rearrange("p (c g) -> p c g",
                                                 g=Gmax)[:, :, 0:Gr],
                            in1=dct[:, g_off:g_off + Gr]
                            .rearrange("p (one g) -> p one g", one=1)
                            .to_broadcast([P, P, Gr]),
                            op=AOT.is_equal)
                        g = pb.tile([P, Gmax, 64], F32, tag="g")
                        nc.gpsimd.dma_gather(
                            g[:, 0:Gr, :],
                            h2t[t * cfg.TROWS:(t + 1) * cfg.TROWS, :],
                            idxw[:, io_off:io_off + L // 16], L, get_reg(L),
                            64, single_packet=False)
                        axp = axps.tile([P, Gmax], F32, space="PSUM",
                                        tag="axp")
                        for gi in range(Gr):
                            b = r["gblocks"][gi]
                            nc.tensor.matmul(
                                axp[:, gi:gi + 1],
                                lhsT=ST[:, gi * P:(gi + 1) * P],
                                rhs=ald2[:, b:b + 1],
                                start=True, stop=True)
                        z = pbs.tile([P, Gmax], F32, tag="z")
                        nc.vector.tensor_tensor(
                            out=z[:, 0:Gr], in0=g[:, 0:Gr, 2],
                            in1=axp[:, 0:Gr], op=AOT.add)
                        nc.vector.scalar_tensor_tensor(
                            out=z[:, 0:Gr], in0=z[:, 0:Gr],
                            scalar=NEG_SLOPE, in1=z[:, 0:Gr],
                            op0=AOT.mult, op1=AOT.max)
                        wb_t = pbs.tile([P, Gmax], BF16, tag="wb")
                        nc.scalar.activation(wb_t[:, 0:Gr], z[:, 0:Gr],
                                             ACT_F.Exp)
                        msg = pbs.tile([P, Gmax, 3], BF16, tag="msg")
                        nc.vector.tensor_tensor(
                            out=msg[:, 0:Gr, 0:2],
                            in0=g[:, 0:Gr, 0:2],
                            in1=wb_t[:, 0:Gr]
                            .rearrange("p (g one) -> p g one", one=1)
                            .to_broadcast([P, Gr, 2]),
                            op=AOT.mult)
                        nc.vector.tensor_copy(msg[:, 0:Gr, 2],
                                              wb_t[:, 0:Gr])
                        for gi in range(Gr):
                            b = r["gblocks"][gi]
                            last = plan.last_touch.get(b) == (t, gi)
                            nc.tensor.matmul(
                                acc_ap(b - w * WB),
                                lhsT=S[:, :, gi], rhs=msg[:, gi, :],
                                start=False, stop=last,
                                skip_group_check=True)
                    sta = pbo.tile([P, WB, 3], F32, tag="sta")
                    for b in range(WB):
                        nc.vector.tensor_copy(sta[:, b, :], accs[b][:])
                    rec = pbo.tile([P, WB], F32, tag="rec")
                    o2 = pbo.tile([P, WB, 2], BF16, tag="o2")
                    nc.vector.tensor_scalar(
                        out=rec[:], in0=sta[:, :, 2], scalar1=1e-12,
                        scalar2=None, op0=AOT.add)
                    nc.vector.reciprocal(rec[:], rec[:])
                    nc.vector.tensor_tensor(
                        out=o2[:], in0=sta[:, :, 0:2],
                        in1=rec[:].rearrange("p (b one) -> p b one", one=1)
                        .to_broadcast([P, WB, 2]), op=AOT.mult)
                    for b in range(WB):
                        blk = w * WB + b
                        nc.tensor.matmul(
                            gacc[:], lhsT=sel_t[:, blk * G:(blk + 1) * G],
                            rhs=o2[:, b, :], start=(blk == 0),
                            stop=(blk == cfg.BLOCKS - 1),
                            skip_group_check=True)
                po = cpool.tile([G, 2], F32)
                nc.vector.tensor_copy(po[:], gacc[:])
                nc.sync.dma_start(part[:, :], po[:])
    nc.compile()
    return nc


# ----------------------------------------------------------------- run glue

_CACHE = {}
LAST_EXEC_NS = None
LAST_EXEC_PARTS = []


def _get_built(cfg, plan):
    key = ("k", cfg.N, cfg.E, plan.key())
    if key not in _CACHE:
        _CACHE[key] = (build_kernel1(cfg, plan), build_kernel2(cfg, plan))
    return _CACHE[key]


def kernel(x, edge_index, batch, W1, a_src1, a_dst1, b1, W2, a_src2, a_dst2,
           b2, cfg=None, use_sim=False):
    global LAST_EXEC_NS, LAST_EXEC_PARTS
    LAST_EXEC_NS = None
    LAST_EXEC_PARTS = []
    cfg = cfg or make_cfg_full()
    x = np.asarray(x, np.float32)
    edge_index = np.asarray(edge_index)
    batch = np.asarray(batch)
    W1 = np.asarray(W1, np.float32)
    W2 = np.asarray(W2, np.float32)
    W1a, W2a = fold_weights(W1, np.asarray(a_src1, np.float32),
                            np.asarray(a_dst1, np.float32), W2,
                            np.asarray(a_src2, np.float32),
                            np.asarray(a_dst2, np.float32), cfg)
    xT = np.zeros((cfg.F_IN, cfg.NPAD), ml_dtypes.bfloat16)
    xT[:, :cfg.N] = x.T.astype(ml_dtypes.bfloat16)
    plan, edges = prep_edges(edge_index, cfg)
    bsels = prep_bsel(batch, cfg)
    ones1 = np.ones((1, P), ml_dtypes.bfloat16)
    iotab = np.tile(np.repeat(np.arange(P, dtype=np.float32), plan.Gmax),
                    (P, 1)).astype(ml_dtypes.bfloat16)
    iotac = np.arange(P, dtype=np.float32).reshape(P, 1)
    identf = np.eye(P, dtype=np.float32)
    identb = np.eye(P, dtype=np.float32).astype(ml_dtypes.bfloat16)

    nc1, nc2 = _get_built(cfg, plan)
    in_maps1 = []
    for k in range(cfg.CORES):
        e = edges[k]
        in_maps1.append({
            "xT": xT, "W1a": W1a, "W2a": W2a,
            "xTo": np.ascontiguousarray(
                xT[:, k * cfg.NODES_PC:(k + 1) * cfg.NODES_PC]),
            "idx": e["idx"], "dcolb": e["dcolb"], "drow": e["drow"],
            "ones1": ones1, "iotab": iotab, "iotac": iotac,
            "identf": identf, "identb": identb,
        })
    res1 = _run(nc1, in_maps1, cfg, use_sim)
    h2aug_full = np.concatenate([r["h2aug"] for r in res1], 0)  # [NPAD, 4]

    h2t = np.zeros((cfg.NPAD, 64), np.float32)
    h2t[:, :4] = h2aug_full
    in_maps2 = []
    for k in range(cfg.CORES):
        e = edges[k]
        own = h2aug_full[k * cfg.NODES_PC:(k + 1) * cfg.NODES_PC]
        own4 = np.ascontiguousarray(
            own.reshape(cfg.BLOCKS, P, 4).transpose(1, 0, 2)
            .reshape(P, cfg.BLOCKS * 4))
        in_maps2.append({
            "h2t": h2t, "own4": own4,
            "idx": e["idx"], "dcolb": e["dcolb"], "drow": e["drow"],
            "ones1": ones1, "iotab": iotab, "iotac": iotac,
            "identb": identb, "bsel": bsels[k],
        })
    res2 = _run(nc2, in_maps2, cfg, use_sim)
    sums = np.sum([r["part"] for r in res2], axis=0)   # [G, 2]
    cnt = np.bincount(np.asarray(batch, np.int64), minlength=cfg.NUM_GRAPHS)
    cnt = np.maximum(cnt, 1).astype(np.float32)[:, None]
    pooled = sums / cnt
    lse = np.log(np.sum(np.exp(pooled - pooled.max(1, keepdims=True)), 1,
                        keepdims=True)) + pooled.max(1, keepdims=True)
    return (pooled - lse).astype(np.float32)


def _run(nc, in_maps, cfg, use_sim):
    global LAST_EXEC_NS
    if use_sim:
        from concourse.bass_interp import CoreSim
        outs = []
        for im in in_maps:
            sim = CoreSim(nc, trace=False, require_finite=False,
                          require_nnan=False)
            for k, v in im.items():
                sim.tensor(k)[:] = v
            sim.simulate(check_with_hw=False)
            names = [a.memorylocations[0].name
                     for a in nc.m.functions[0].allocations
                     if getattr(a, "kind", None) == "ExternalOutput"]
            outs.append({n: np.array(sim.tensor(n)) for n in names})
        return outs
    from concourse.bass_utils import run_bass_kernel_spmd
    kwargs = {}
    tdir = os.environ.get("GAT_TRACE_DIR")
    if tdir:
        sub = os.path.join(tdir, f"launch{len(LAST_EXEC_PARTS)}")
        os.makedirs(sub, exist_ok=True)
        kwargs["tmpdir"] = sub
    r = run_bass_kernel_spmd(nc, in_maps, core_ids=list(range(cfg.CORES)),
                             **kwargs)
    if r.exec_time_ns is not None:
        LAST_EXEC_PARTS.append(r.exec_time_ns)
        LAST_EXEC_NS = int(sum(LAST_EXEC_PARTS))
    return r.results

